# revision 1
# baseline (speedup 1.0000x reference)
"""Distributed Trainium2 kernel for nn_CompareLoss (8 NeuronCores).

Math (validated against the reference):
  z = [strong; weak]  (2B x D), s = z / ||z||  (row-normalized)
  logits(i,j) = (s_i . s_j) / tau,  pos_i = logits(i, B+i) = logits(B+i, i)
  Every row r of the similarity matrix contributes  ln(S_r) - pos_r  where
    S_r = exp(pos_r) + sum_{j in C(r)} exp(logits(r, j))
  with column set C(r):
    - "positive" rows (strong_i / weak_{B+i}, i < P): C = all 2N negative rows
    - "negative" rows (i >= P):                       C = the P strong-positive rows
  loss = (sum over all 2B rows) / (2B).
  Logits are bounded by 1/tau, so no max-subtraction is needed in the LSE.

Sharding: data-parallel over the pair index i. Core c owns i in
[c*256,(c+1)*256) of the positives AND of the negatives -> 1024 rows/core,
perfectly balanced work. Each core receives the full column set
feature-major (z^T) with its own row blocks rotated to the front of each
region so one SPMD program (fixed offsets) serves all 8 cores. No
collectives: on this fabric an 8-rank collective has a ~7-20us floor,
far more than host-summing 8 scalar partials.

On-device pipeline (all aux matmuls in fp16 - fp32 matmuls run LOW_HIGH
double-pass on TRN2 and must be avoided):
  A) column sum-of-squares: per 512-col chunk square z^T (G1 chunks on the
     vector engine, G2 on gpsimd) and partition-sum with a fp16 ones-matmul
     into batched [1,2048] psum tiles; one fused ACT Ln moves each batch
     psum->sbuf into a flat [1,W] staging buffer.
  B) rn = exp(-0.5*ln(ssq) + 0.5*ln(1/tau)) evaluated directly on the flat
     buffer (a ~3us single-partition ACT pass beats any reshape-DMA round
     trip); the rsqrt and the sqrt(1/tau) logit scale fold into one pass,
     and Ln/Exp phases are separated so the ACT table loads stay rare.
  C) rn broadcast to all partitions via a stride-0 DRAM-read DMA into a
     column-indexed SBUF buffer, then vector-engine scale multiplies (f16
     2x mode) -> fp16 ztn. Fully PSUM-free: the psum pool serves only the
     sumsq batches and the main jobs, minimizing 2-slot rotation gating.
     G1 (M2-critical columns) is processed first.
  z^T is passed fp16 from the host (the similarity math is fp16 anyway):
  halves the HBM-roofline input DMA and doubles DVE multiply throughput.
  Mains: 12 jobs of [128,2048]: 8 fp16 matmuls + one Exp with fused
  row-sum (activation accum_out). ln(S)-pos is reduced on-chip to a single
  f32 partial per core; the host adds 8 partials and divides by 2B.
"""

import numpy as np

B = 4096
D = 256
P = 2048
NCORES = 8
IC = P // NCORES          # 256 pair-indices per core (per pos/neg half)
NCOL = 3 * P + IC         # 6400 columns in zt: [sneg | wneg | spos | wp_c]

OFF_SNEG = 0
OFF_WNEG = P
OFF_SPOS = 2 * P
OFF_WP = 3 * P
# lhsT column offsets for the 8 m-tiles (128 rows each):
#   M1 (positive rows): sp0 sp1 wp0 wp1      M2 (negative rows): sn0 sn1 wn0 wn1
LHS_OFF = [OFF_SPOS, OFF_SPOS + 128, OFF_WP, OFF_WP + 128,
           OFF_SNEG, OFF_SNEG + 128, OFF_WNEG, OFF_WNEG + 128]
POS_COL = [0, 1, 0, 1, 2, 3, 2, 3]   # pos i-tile used by each m-tile

# normalization chunks: G1 = everything the M2 jobs + pos logits + all lhsT
# tiles need; G2 = the M1 rhs columns. Widths are multiples of 128.
G1_CHUNKS = [(4096, 512), (4608, 512), (5120, 512), (5632, 512), (6144, 256),
             (0, 256), (2048, 256)]
G2_CHUNKS = [(256, 512), (768, 512), (1280, 512), (1792, 256),
             (2304, 512), (2816, 512), (3328, 512), (3840, 256)]
CHUNKS = G1_CHUNKS + G2_CHUNKS
N_ACT_PATH = len(G1_CHUNKS)   # G1 chunks move psum->sbuf via fused ACT Ln,
                              # G2 chunks via DVE copy (ln'd later compactly)

_CACHE: dict = {}


def _build_nc():
    import concourse.bacc as bacc
    import concourse.tile as tile
    from concourse import mybir

    f32 = mybir.dt.float32
    f16 = mybir.dt.float16
    EXP = mybir.ActivationFunctionType.Exp
    LN = mybir.ActivationFunctionType.Ln
    AX = mybir.AxisListType.X
    ADD = mybir.AluOpType.add

    nc = bacc.Bacc("TRN2", target_bir_lowering=False, debug=False,
                   num_devices=NCORES)
    zt_d = nc.dram_tensor("zt", [D, NCOL], f16, kind="ExternalInput")
    tp_d = nc.dram_tensor("temp", [1, 1], f32, kind="ExternalInput")
    out_d = nc.dram_tensor("out", [1, 1], f32, kind="ExternalOutput")

    # column offset of each chunk inside the compact [128, 50] layout
    coffs, acc = [], 0
    for _, w in CHUNKS:
        coffs.append(acc)
        acc += w // 128
    NCC = acc  # 50

    with tile.TileContext(nc) as tc:
        with (
            tc.tile_pool(name="const", bufs=1) as constp,
            tc.tile_pool(name="big", bufs=1) as bigp,
            tc.tile_pool(name="work", bufs=3) as workp,
            tc.tile_pool(name="esc", bufs=2) as escp,
            tc.tile_pool(name="dram", bufs=1, space="DRAM") as dramp,
            tc.tile_pool(name="ps", bufs=2, space="PSUM") as psp,
        ):
            # ---------------- load z^T (G1 columns first) ----------------
            # Groups are chained (group k waits on group k-1) so the DMA
            # engines complete them in chunk-processing order instead of
            # round-robining all ranges to a late joint finish.
            from concourse.tile_rust import add_dep_helper
            zt0 = bigp.tile([128, NCOL], f16)        # features 0:128
            zt1 = bigp.tile([128, NCOL], f16)        # features 128:256
            # group 0 (all of G1) runs at full bandwidth; G2 chains behind
            dma_groups = [
                [(4096, 2304), (0, 256), (2048, 256)],
                [(256, 1792), (2304, 1792)],
            ]
            prev = []
            for grp in dma_groups:
                cur = []
                for s, w in grp:
                    cur.append(nc.sync.dma_start(zt0[:, s:s + w],
                                                 zt_d[0:128, s:s + w]))
                    cur.append(nc.sync.dma_start(zt1[:, s:s + w],
                                                 zt_d[128:D, s:s + w]))
                for a in cur:
                    for b in prev:
                        add_dep_helper(a.ins, b.ins, sync=True,
                                       reason="dma group ordering")
                prev = cur

            # ---------------- constants ----------------
            ones16_k = constp.tile([128, 1], f16)    # fp16 partition-sum
            nc.gpsimd.memset(ones16_k[:], 1.0)
            ones16_1 = constp.tile([1, 128], f16)    # fp16 partition-broadcast
            nc.gpsimd.memset(ones16_1[:], 1.0)
            ident16 = constp.tile([1, 1], f16)
            nc.gpsimd.memset(ident16[:], 1.0)
            ones_k = constp.tile([128, 1], f32)      # final f32 total-sum
            nc.gpsimd.memset(ones_k[:], 1.0)

            tsb = constp.tile([1, 1], f32)
            nc.sync.dma_start(tsb[:], tp_d[:])
            invt = constp.tile([1, 1], f32)
            nc.vector.reciprocal(invt[:], tsb[:])
            ln_invt = constp.tile([1, 1], f32)
            nc.scalar.activation(ln_invt[:], invt[:], LN)
            half_ln_invt = constp.tile([1, 1], f32)
            nc.scalar.mul(half_ln_invt[:], ln_invt[:], 0.5)
            # broadcast 0.5*ln(1/tau) to [128,1] via a K=1 matmul (NOT
            # gpsimd.partition_broadcast - its custom ucode forces a GPSIMD
            # library switch that stalls the engine for ~15us)
            hli16 = constp.tile([1, 1], f16)
            nc.vector.tensor_copy(hli16[:], half_ln_invt[:])
            bias_ps = psp.tile([128, 1], f32, tag="ps")
            nc.tensor.matmul(bias_ps[:], ones16_1[:], hli16[0:1, 0:1],
                             start=True, stop=True)
            bias_bc = constp.tile([128, 1], f32)     # 0.5*ln(1/tau) everywhere
            nc.vector.tensor_copy(bias_bc[:], bias_ps[:])


            # ---------------- A/B/C: column normalization ----------------
            # Per chunk: squares -> ones-matmul partition sum -> [1,w] psum,
            # moved to a flat [1,*] staging buffer (G1 via fused ACT
            # Ln-from-psum, G2 via DVE copy). One reshape DMA per group
            # gives a compact [128,*] layout for the rsqrt, whose fp16
            # result is reshaped back and PE-broadcast per chunk for the
            # scale multiply. G1 completes first so the M2 similarity jobs
            # and pos logits can start while G2 is still streaming in.
            ztn0 = bigp.tile([128, NCOL], f16)
            ztn1 = bigp.tile([128, NCOL], f16)
            WA = sum(w for _, w in CHUNKS[:N_ACT_PATH])
            WB = sum(w for _, w in CHUNKS[N_ACT_PATH:])
            goffs, oa, ob = [], 0, 0
            for ci, (_, w) in enumerate(CHUNKS):
                if ci < N_ACT_PATH:
                    goffs.append(("A", oa)); oa += w
                else:
                    goffs.append(("B", ob)); ob += w
            flatA = constp.tile([1, WA], f32)
            flatB = constp.tile([1, WB], f32)

            def sumsq_batch(cis):
                """sumsq for a run of chunks into ONE [1, <=2048] psum tile
                (one pool-slot allocation + one big Ln instead of per-chunk
                ones - the 2-slot psum rotation is a serializer otherwise)."""
                grp, go0 = goffs[cis[0]]
                flat = flatA if grp == "A" else flatB
                wtot = sum(CHUNKS[ci][1] for ci in cis)
                assert wtot <= 2048
                ss_ps = psp.tile([1, 2048], f32, tag="ps",
                                 name=f"ssb{cis[0]}")
                o = 0
                for ci in cis:
                    s, w = CHUNKS[ci]
                    # G1 squares both on DVE (earliest data, keeps its FIFO
                    # short ahead of the C(G1) multiplies); G2 both on GPSIMD
                    eng = nc.vector if grp == "A" else nc.gpsimd
                    sq0 = workp.tile([128, 512], f16, tag="sq0",
                                     name=f"sq0_{ci}")
                    sq1 = workp.tile([128, 512], f16, tag="sq1",
                                     name=f"sq1_{ci}")
                    eng.tensor_mul(sq0[:, :w], zt0[:, s:s + w],
                                   zt0[:, s:s + w])
                    eng.tensor_mul(sq1[:, :w], zt1[:, s:s + w],
                                   zt1[:, s:s + w])
                    nc.tensor.matmul(ss_ps[0:1, o:o + w], ones16_k[:],
                                     sq0[:, :w], start=True, stop=False)
                    nc.tensor.matmul(ss_ps[0:1, o:o + w], ones16_k[:],
                                     sq1[:, :w], start=False, stop=True)
                    o += w
                # fused psum->sbuf move + ln on the scalar engine
                nc.scalar.activation(flat[0:1, go0:go0 + wtot],
                                     ss_ps[0:1, :wtot], LN)

            # rn broadcast lives in SBUF (column-indexed, written by DMA
            # with a stride-0 DRAM source) - PSUM-free, so the psum pool
            # serves only the sumsq batches and the main jobs, and the f16
            # multiplies run in the DVE 2x mode
            rnb = bigp.tile([128, NCOL], f16)

            def norm_chunk(ci, _rnflat=None):
                s, w = CHUNKS[ci]
                nc.vector.tensor_mul(ztn0[:, s:s + w], zt0[:, s:s + w],
                                     rnb[:, s:s + w])
                nc.vector.tensor_mul(ztn1[:, s:s + w], zt1[:, s:s + w],
                                     rnb[:, s:s + w])

            # --- G1: sumsq -> rsqrt-on-flat -> scale; unblocks M2 early ---
            sumsq_batch([0, 1, 2, 3])
            sumsq_batch([4, 5, 6])
            # rn = exp(-0.5*ln(ssq)+0.5*ln(1/tau)) computed directly on the
            # flat [1,W] buffer: one single-partition ACT pass costs ~2.5us,
            # far less than the reshape-DMA round trip it replaces
            rnflatA = constp.tile([1, WA], f16)
            nc.scalar.activation(rnflatA[0:1, :], flatA[0:1, :], EXP,
                                 scale=-0.5, bias=bias_bc[0:1, 0:1])
            rnfA = dramp.tile([1, WA], f16)
            nc.scalar.dma_start(rnfA[0:1, :], rnflatA[0:1, :])
            # (flat offset, column, width); small M2-lhsT runs first
            for fo, s, w in [(2304, 0, 256), (2560, 2048, 256),
                             (0, 4096, 2304)]:
                nc.sync.dma_start(rnb[:, s:s + w],
                                  rnfA[0:1, fo:fo + w].to_broadcast((128, w)))
            # The G2 sumsq batches are interleaved BETWEEN the C(G1)
            # broadcasts: their psum-slot grants (grant k <- release k-2 in
            # the 2-slot rotation) then land after the M2-critical chunks
            # instead of gating them; their matmuls only need data when the
            # gpsimd squares finish (~33us) anyway.
            for ci in [5, 6, 0, 1]:
                norm_chunk(ci, rnflatA)
            sumsq_batch([7, 8, 9, 10])
            for ci in [2, 3]:
                norm_chunk(ci, rnflatA)
            sumsq_batch([11, 12, 13, 14])
            norm_chunk(4, rnflatA)

            # ---------------- pos logits (all columns in G1) ---------------
            pos_ps = psp.tile([1, 512], f32, tag="ps")
            for half, (ca, cb) in enumerate(
                    [(OFF_SPOS, OFF_WP), (OFF_SNEG, OFF_WNEG)]):
                pr0 = workp.tile([128, IC], f16, tag="pr0")
                pr1 = workp.tile([128, IC], f16, tag="pr1")
                nc.vector.tensor_mul(pr0[:], ztn0[:, ca:ca + IC],
                                     ztn0[:, cb:cb + IC])
                nc.vector.tensor_mul(pr1[:], ztn1[:, ca:ca + IC],
                                     ztn1[:, cb:cb + IC])
                o = half * 2 * 128
                nc.tensor.matmul(pos_ps[0:1, o:o + IC], ones16_k[:], pr0[:],
                                 start=True, stop=False)
                nc.tensor.matmul(pos_ps[0:1, o:o + IC], ones16_k[:], pr1[:],
                                 start=False, stop=True)
            pos_sb = constp.tile([1, 512], f32)
            nc.vector.tensor_copy(pos_sb[:], pos_ps[:])
            pos16 = constp.tile([1, 512], f16)
            nc.vector.tensor_copy(pos16[:], pos_sb[:])

            # transpose pos to per-partition layout via [1,128]x[1,1] matmuls
            P_mat = constp.tile([128, 8], f32)
            for t in range(4):
                pos_t = psp.tile([128, 1], f32, tag="ps", name=f"pt{t}")
                nc.tensor.matmul(pos_t[:], pos16[0:1, t * 128:(t + 1) * 128],
                                 ident16[0:1, 0:1], start=True, stop=True)
                for col in range(8):
                    if POS_COL[col] == t:
                        nc.vector.tensor_copy(P_mat[:, col:col + 1], pos_t[:])

            # ---------------- main similarity jobs ----------------
            # 12 jobs of [128, 2048]: M1 m-tiles have 2 jobs (4096 cols),
            # M2 m-tiles have 1 (2048 cols). ACC col: M1 -> mt*2+j, M2 -> 8+mt.
            ACC = constp.tile([128, 12], f32)

            def main_job(mt, j, acccol):
                off = LHS_OFF[mt]
                js = (0 if mt < 4 else 2 * P) + j * 2048
                ps = psp.tile([128, 2048], f32, tag="ps", name=f"mm{acccol}")
                for h in range(4):
                    c0 = js + h * 512
                    nc.tensor.matmul(ps[:, h * 512:(h + 1) * 512],
                                     ztn0[:, off:off + 128],
                                     ztn0[:, c0:c0 + 512],
                                     start=True, stop=False)
                    nc.tensor.matmul(ps[:, h * 512:(h + 1) * 512],
                                     ztn1[:, off:off + 128],
                                     ztn1[:, c0:c0 + 512],
                                     start=False, stop=True)
                # exp with fused row-sum (SBUF dst: in-place psum writes
                # contend with the psum read port)
                esc = escp.tile([128, 2048], f16, tag="esc",
                                name=f"esc{acccol}")
                nc.scalar.activation(esc[:], ps[:], EXP,
                                     accum_out=ACC[:, acccol:acccol + 1])

            # --- G2 rsqrt + broadcast via DRAM (PSUM-free so it cannot
            # contend with the main jobs' psum slots; enables gpsimd mults)
            rnflatB = constp.tile([1, WB], f16)
            nc.scalar.activation(rnflatB[0:1, :], flatB[0:1, :], EXP,
                                 scale=-0.5, bias=bias_bc[0:1, 0:1])
            rnfB = dramp.tile([1, WB], f16)
            nc.scalar.dma_start(rnfB[0:1, :], rnflatB[0:1, :])
            for fo, s, w in [(0, 256, 1792), (1792, 2304, 1792)]:
                nc.sync.dma_start(rnb[:, s:s + w],
                                  rnfB[0:1, fo:fo + w].to_broadcast((128, w)))

            # M2 jobs (need only G1 columns) overlap C(G2)
            for i in range(4):
                main_job(4 + i, 0, 8 + i)
            # C(G2): two large scale-multiplies per k-half, split DVE/gpsimd,
            # interleaved with the M1 jobs that consume them
            nc.vector.tensor_mul(ztn0[:, 256:2048], zt0[:, 256:2048],
                                 rnb[:, 256:2048])
            nc.vector.tensor_mul(ztn1[:, 256:2048], zt1[:, 256:2048],
                                 rnb[:, 256:2048])
            for mt in range(4):
                main_job(mt, 0, mt * 2)
            nc.vector.tensor_mul(ztn0[:, 2304:4096], zt0[:, 2304:4096],
                                 rnb[:, 2304:4096])
            nc.vector.tensor_mul(ztn1[:, 2304:4096], zt1[:, 2304:4096],
                                 rnb[:, 2304:4096])
            for mt in range(4):
                main_job(mt, 1, mt * 2 + 1)

            # ---------------- reduce & finish ----------------
            # E_mat emitted here so its ACT slot doesn't head-of-line-block
            # the G2 Ln ops behind the pos-logit dependency
            E_mat = constp.tile([128, 8], f32)
            nc.scalar.activation(E_mat[:], P_mat[:], EXP)
            RS = constp.tile([128, 8], f32)
            nc.vector.tensor_reduce(
                RS[:, 0:4], ACC[:, 0:8].rearrange("p (m j) -> p m j", j=2),
                axis=AX, op=ADD)
            nc.vector.tensor_copy(RS[:, 4:8], ACC[:, 8:12])
            S_mat = constp.tile([128, 8], f32)
            nc.vector.tensor_add(S_mat[:], RS[:], E_mat[:])
            LnS = constp.tile([128, 8], f32)
            nc.scalar.activation(LnS[:], S_mat[:], LN)
            Dif = constp.tile([128, 8], f32)
            nc.vector.tensor_sub(Dif[:], LnS[:], P_mat[:])
            part = constp.tile([128, 1], f32)
            nc.vector.tensor_reduce(part[:], Dif[:], axis=AX, op=ADD)
            tot_ps = psp.tile([1, 1], f32, tag="ps")
            nc.tensor.matmul(tot_ps[0:1, 0:1], ones_k[:], part[:],
                             start=True, stop=True)
            out_sb = constp.tile([1, 1], f32)
            nc.vector.tensor_copy(out_sb[:], tot_ps[:])
            nc.sync.dma_start(out_d[:], out_sb[:])

    nc.compile()
    return nc


def get_nc():
    if "nc" not in _CACHE:
        _CACHE["nc"] = _build_nc()
    return _CACHE["nc"]


def make_in_maps(strong: np.ndarray, weak: np.ndarray, temp: np.ndarray):
    """Host-side sharding: slice + rotate + transpose (pure data movement)."""
    in_maps = []
    for c in range(NCORES):
        r = c * IC
        sneg = np.roll(strong[P:B], -r, axis=0)   # own sn_c first
        wneg = np.roll(weak[P:B], -r, axis=0)     # own wn_c first
        spos = np.roll(strong[0:P], -r, axis=0)   # own sp_c first
        wp = weak[r:r + IC]
        zt = np.ascontiguousarray(
            np.concatenate([sneg, wneg, spos, wp], axis=0).T.astype(np.float16))
        in_maps.append({"zt": zt, "temp": temp})
    return in_maps


def kernel(inputs, strong_inputs, targets, num_pos, temperature):
    assert int(num_pos) == P
    strong = np.ascontiguousarray(np.asarray(strong_inputs, dtype=np.float32))
    weak = np.ascontiguousarray(np.asarray(inputs, dtype=np.float32))
    temp = np.asarray(temperature, dtype=np.float32).reshape(1, 1)

    from concourse.bass_utils import run_bass_kernel_spmd

    nc = get_nc()
    in_maps = make_in_maps(strong, weak, temp)
    res = run_bass_kernel_spmd(nc, in_maps, core_ids=list(range(NCORES)))
    total = sum(float(np.asarray(r["out"]).reshape(-1)[0])
                for r in res.results)
    return np.float32(total / (2 * B))



# revision 12
# speedup vs baseline: 1.1951x; 1.1951x over previous
"""Distributed Trainium2 kernel for nn_CompareLoss (8 NeuronCores).

Math (validated against the reference):
  z = [strong; weak]  (2B x D), s = z / ||z||  (row-normalized)
  logits(i,j) = (s_i . s_j) / tau,  pos_i = logits(i, B+i) = logits(B+i, i)
  Every row r contributes  ln(e^{pos_r} + sum_{j in C(r)} e^{logits(r,j)})
  - pos_r, with column set C(r):
    - positive rows (strong_i / weak_{B+i}, i < P): C = all 2N negative rows
    - negative rows (i >= P):                       C = the P strong-pos rows
  loss = (sum over all 2B rows) / (2B).  Logits bounded by 1/tau -> no
  max-subtraction needed in the LSE.

Sharding: data-parallel over the pair index. Core c owns 256 positive and
256 negative pairs -> 1024 rows/core.  Each core gets the full column set
feature-major (z^T, fp16) with its own row blocks rotated to the front so
one SPMD program serves all 8 cores.  No collectives; the host sums 8
tiny partial tensors.

Device pipeline (engine-balanced around the ACT exp floor of ~24us):
  - zt loads stream in 8 chunked DMAs; the M2-critical prefix
    [sn|wn|spos|wp] (2816 cols) first.
  - Column norms: DVE squares (one 3D [128,2,w] op per chunk) -> paired
    ones-matmuls whose lhsT VALUE is 1/tau (runtime, broadcast via a K=1
    matmul), accumulating invtau*ssq into a multi-partition [rows,512]
    psum tile -> DVE reciprocal -> one tiny ACT Sqrt -> rn rows, written
    flat to DRAM once and partition-broadcast back in 3+4 wide DMAs.
    This keeps ACT's non-exp work ~1.5us total (the baseline spent ~17us
    in single-partition Ln/Exp passes + 5 table swaps).
  - 12 main jobs [128,2048]: 8 fp16 matmuls + one ACT Exp with fused
    row-sum (accum_out).  The exp outputs are written to one write-only
    scratch tile; only the row sums (ACC[128,12]) leave the device.
  - Raw pos-pair logits ([1,512] psum) ship to the host as well; the
    host does the final ln(S + e^pos) - pos reduction in float64 (cheap:
    12K values/core), removing the tail Ln + its table load.
  - PE is kept continuously busy from ~6.5us (junk warm-up matmuls) so
    the tensor engine reaches its max p-state before the real matmuls.
  - DMA triggers cost ~600ns each on the issuing queue: zt + broadcast +
    output triggers all live on the Sync queue (idle otherwise); the
    Scalar queue runs pure ACT so exps are never blocked behind DMA.
  - PSUM is exactly 8 banks: every psum tile shares one 2-slot pool
    ([128,2048] slots); iv/ssqA/ssqB rotate through before the mains.
"""

import numpy as np

B = 4096
D = 256
P = 2048
NCORES = 8
IC = P // NCORES          # 256 pair-indices per core (per pos/neg half)
NCOL = 3 * P + IC         # 6400 columns

# column layout: [sn 256 | wn 256 | spos 2048 | wp 256 | snr 1792 | wnr 1792]
OFF_SN = 0
OFF_WN = 256
OFF_SPOS = 512
OFF_WP = 2560
OFF_SNR = 2816
OFF_WNR = 4608

# zt DMA chunks (start, width): M2-critical prefix first, then G2
ZT_CHUNKS = [(0, 512), (512, 1024), (1536, 1024), (2560, 256),
             (2816, 1024), (3840, 1024), (4864, 1024), (5888, 512)]
N_G1 = 4                  # first 4 chunks = G1 (cols 0:2816)

# ssq rows: 512 columns per psum partition row
G1_COLS = 2816            # ssA rows 0..5 (row5 cols 0:256 used)
G2_COLS = 3584            # ssB rows 0..6

# rn broadcast ranges (dst start, width, flat tensor, flat offset)
BCA = [(0, 1024, 0), (1024, 1024, 1024), (2048, 768, 2048)]
BCB = [(2816, 1024, 0), (3840, 1024, 1024), (4864, 1024, 2048),
       (5888, 512, 3072)]

# main jobs: (lhsT col, [4 rhs 512-col slice starts], ACC col)
M2_JOBS = [(OFF_SN, [512, 1024, 1536, 2048], 0),
           (OFF_SN + 128, [512, 1024, 1536, 2048], 1),
           (OFF_WN, [512, 1024, 1536, 2048], 2),
           (OFF_WN + 128, [512, 1024, 1536, 2048], 3)]
# the 8 negative 512-col slices: [0:512] = sn+wn, the rest = snr+wnr.
# Each M1 lhsT tile covers slice-set A in its first job and B in its
# second (order within a job is irrelevant - only the row-sum is kept).
NEG_A = [0, 2816, 3328, 3840]
NEG_B = [4352, 4864, 5376, 5888]
M1_JOBS = [(OFF_SPOS, NEG_A, 4),
           (OFF_SPOS + 128, NEG_A, 6),
           (OFF_WP, NEG_A, 8),
           (OFF_WP + 128, NEG_A, 10)]
M1_JOBS_2 = [(OFF_SPOS, NEG_B, 5),
             (OFF_SPOS + 128, NEG_B, 7),
             (OFF_WP, NEG_B, 9),
             (OFF_WP + 128, NEG_B, 11)]

_CACHE: dict = {}


def _build_nc():
    import concourse.bacc as bacc
    import concourse.tile as tile
    from concourse import mybir
    from concourse.tile_rust import add_dep_helper

    f32 = mybir.dt.float32
    f16 = mybir.dt.float16
    EXP = mybir.ActivationFunctionType.Exp
    SQRT = mybir.ActivationFunctionType.Sqrt

    nc = bacc.Bacc("TRN2", target_bir_lowering=False, debug=False,
                   num_devices=NCORES)
    zt_d = nc.dram_tensor("zt", [2 * D // 2, NCOL], f16, kind="ExternalInput")
    iv_d = nc.dram_tensor("invtau", [1, 1], f32, kind="ExternalInput")
    acc_d = nc.dram_tensor("acc", [128, 12], f32, kind="ExternalOutput")
    pos_d = nc.dram_tensor("pos", [1, 512], f32, kind="ExternalOutput")

    zt3 = zt_d[:, :].rearrange("(h p) c -> p h c", h=2)  # [128,2,NCOL] view

    with tile.TileContext(nc) as tc:
        with (
            tc.tile_pool(name="const", bufs=1) as constp,
            tc.tile_pool(name="big", bufs=1) as bigp,
            tc.tile_pool(name="work", bufs=3) as workp,
            tc.tile_pool(name="dram", bufs=1, space="DRAM") as dramp,
            tc.tile_pool(name="ps", bufs=2, space="PSUM") as psp,
        ):
            # ---------------- input DMAs (sync queue) ----------------
            zt = bigp.tile([128, 2, NCOL], f16)
            ivt = constp.tile([1, 1], f32)
            d_iv = nc.sync.dma_start(ivt[:], iv_d[:])
            prev = [d_iv]
            for s, w in ZT_CHUNKS:
                d = nc.sync.dma_start(zt[:, :, s:s + w], zt3[:, :, s:s + w])
                # chain with depth 2 so chunks arrive in order at full
                # bandwidth instead of round-robining to a joint finish
                if len(prev) >= 2:
                    add_dep_helper(d.ins, prev[-2].ins, sync=True,
                                   reason="zt chunk ordering")
                prev.append(d)

            # ---------------- constants ----------------
            ones16_1 = constp.tile([1, 128], f16)
            nc.gpsimd.memset(ones16_1[:], 1.0)
            ones16_k = constp.tile([128, 1], f16)
            nc.gpsimd.memset(ones16_k[:], 1.0)
            junkW = constp.tile([128, 128], f16)
            nc.gpsimd.memset(junkW[:], 0.0)
            junkR = constp.tile([128, 1], f16)
            nc.gpsimd.memset(junkR[:], 0.0)

            # invtau broadcast to [128,1] via K=1 matmul.  EZ is the
            # sumsq lhsT: sliding 13-col windows of [0*13 | invtau | 0*12]
            # place invtau*ssq of chunk r at psum partition r (matmul
            # output must start at partition 0, so the row index comes
            # from the hot column's position inside the window).
            iv16 = constp.tile([1, 1], f16)
            nc.vector.tensor_copy(iv16[:], ivt[:])
            iv_ps = psp.tile([128, 1], f32, tag="ps")
            nc.tensor.matmul(iv_ps[:], ones16_1[0:1, :], iv16[0:1, 0:1],
                             start=True, stop=True)
            EZ = constp.tile([128, 26], f16)
            nc.gpsimd.memset(EZ[:], 0.0)
            nc.vector.tensor_copy(EZ[:, 13:14], iv_ps[:])

            # PE warm-up: keep the tensor engine continuously busy so it
            # ramps to max p-state before the real matmuls (ldweights of
            # a [128,128] tile dominates; results overwrite iv_ps, dead)
            for _ in range(12):
                nc.tensor.matmul(iv_ps[:, 0:1], junkW[:], junkR[0:128, 0:1],
                                 start=True, stop=True)

            # ---------------- column sumsq -> rn ----------------
            # squares per chunk (one 3D DVE op), then EZ-window matmuls
            # accumulating invtau*ssq of 512-col subchunk r into psum
            # partition r of a single [13,512] tile (1 bank).  The first
            # matmul (start=True, full width) zero-fills all rows, so the
            # reciprocal never reads uninitialized psum.
            ssA = psp.tile([13, 512], f32, tag="ps")
            ssB = psp.tile([13, 512], f32, tag="ps")

            def do_chunk(s, w, grp_base, ss, first, last):
                sq = workp.tile([128, 2, 1024], f16, tag="sq",
                                name=f"sq_{s}")
                nc.vector.tensor_mul(sq[:, :, 0:w], zt[:, :, s:s + w],
                                     zt[:, :, s:s + w])
                lo = 0
                while lo < w:
                    r = (s - grp_base + lo) // 512
                    ww = min(512, w - lo)
                    for h in range(2):
                        nc.tensor.matmul(
                            ss[0:13, 0:ww], EZ[:, 13 - r:26 - r],
                            sq[:, h, lo:lo + ww],
                            start=(first and lo == 0 and h == 0),
                            stop=(last and lo + ww >= w and h == 1))
                    lo += ww

            for i, (s, w) in enumerate(ZT_CHUNKS[:N_G1]):
                do_chunk(s, w, 0, ssA, i == 0, i == N_G1 - 1)

            # G1 rn: DVE reciprocal -> ACT sqrt -> f16 rows
            rsqA = constp.tile([6, 512], f32)
            nc.vector.reciprocal(rsqA[:], ssA[0:6, :])
            rn_tA = constp.tile([6, 512], f16)
            nc.scalar.activation(rn_tA[:], rsqA[:], SQRT)
            flatA = dramp.tile([1, 3072], f16)
            d_fA = nc.sync.dma_start(
                flatA[0:1, :].rearrange("o (p c) -> p (o c)", p=6), rn_tA[:])

            # G2 chunks (squares + matmuls run as data lands)
            ng2 = len(ZT_CHUNKS) - N_G1
            for i, (s, w) in enumerate(ZT_CHUNKS[N_G1:]):
                do_chunk(s, w, OFF_SNR, ssB, i == 0, i == ng2 - 1)

            rnb = bigp.tile([128, NCOL], f16)
            bcA = []
            for ds, w, fo in BCA:
                d = nc.sync.dma_start(rnb[:, ds:ds + w],
                                      flatA[0:1, fo:fo + w]
                                      .to_broadcast((128, w)))
                add_dep_helper(d.ins, d_fA.ins, sync=True,
                               reason="bcast after flat write")
                bcA.append(d)

            # G1 normalize (DVE), lhsT/pos chunks first via range order
            ztn = bigp.tile([128, 2, NCOL], f16)
            for ds, w, _ in BCA:
                for h in range(2):
                    nc.vector.tensor_mul(ztn[:, h, ds:ds + w],
                                         zt[:, h, ds:ds + w],
                                         rnb[:, ds:ds + w])

            # G2 rn
            rsqB = constp.tile([7, 512], f32)
            nc.vector.reciprocal(rsqB[:], ssB[0:7, :])
            rn_tB = constp.tile([7, 512], f16)
            nc.scalar.activation(rn_tB[:], rsqB[:], SQRT)
            flatB = dramp.tile([1, 3584], f16)
            d_fB = nc.sync.dma_start(
                flatB[0:1, :].rearrange("o (p c) -> p (o c)", p=7), rn_tB[:])
            for ds, w, fo in BCB:
                d = nc.sync.dma_start(rnb[:, ds:ds + w],
                                      flatB[0:1, fo:fo + w]
                                      .to_broadcast((128, w)))
                add_dep_helper(d.ins, d_fB.ins, sync=True,
                               reason="bcast after flat write")

            # ---------------- main similarity jobs ----------------
            ACC = constp.tile([128, 12], f32)
            esc = constp.tile([128, 2048], f16)   # write-only exp sink

            def main_job(lhs_off, rhs_list, acccol):
                ps = psp.tile([128, 2048], f32, tag="ps",
                              name=f"mm{acccol}")
                for h4, c0 in enumerate(rhs_list):
                    for h in range(2):
                        nc.tensor.matmul(
                            ps[:, h4 * 512:(h4 + 1) * 512],
                            ztn[:, h, lhs_off:lhs_off + 128],
                            ztn[:, h, c0:c0 + 512],
                            start=(h == 0), stop=(h == 1))
                nc.scalar.activation(esc[:], ps[:], EXP,
                                     accum_out=ACC[:, acccol:acccol + 1])

            for lhs_off, rhs_list, acccol in M2_JOBS:
                main_job(lhs_off, rhs_list, acccol)

            # G2 normalize interleaves with the M2 jobs on the DVE
            for ds, w, _ in BCB:
                for h in range(2):
                    nc.vector.tensor_mul(ztn[:, h, ds:ds + w],
                                         zt[:, h, ds:ds + w],
                                         rnb[:, ds:ds + w])

            for lhs_off, rhs_list, acccol in M1_JOBS:
                main_job(lhs_off, rhs_list, acccol)
            for lhs_off, rhs_list, acccol in M1_JOBS_2:
                main_job(lhs_off, rhs_list, acccol)

            # ---------------- raw pos-pair logits ----------------
            # products of normalized columns; summed over k by ones-matmul
            pr_pos = workp.tile([128, 2, 256], f16, tag="pr")
            nc.vector.tensor_mul(pr_pos[:],
                                 ztn[:, :, OFF_SPOS:OFF_SPOS + 256],
                                 ztn[:, :, OFF_WP:OFF_WP + 256])
            pr_neg = workp.tile([128, 2, 256], f16, tag="pr")
            nc.vector.tensor_mul(pr_neg[:],
                                 ztn[:, :, OFF_SN:OFF_SN + 256],
                                 ztn[:, :, OFF_WN:OFF_WN + 256])
            pos_ps = psp.tile([1, 512], f32, tag="ps")
            for half, pr in ((0, pr_pos), (1, pr_neg)):
                o = half * 256
                nc.tensor.matmul(pos_ps[0:1, o:o + 256], ones16_k[:],
                                 pr[:, 0, :], start=True, stop=False)
                nc.tensor.matmul(pos_ps[0:1, o:o + 256], ones16_k[:],
                                 pr[:, 1, :], start=False, stop=True)
            pos_sb = constp.tile([1, 512], f32)
            nc.vector.tensor_copy(pos_sb[:], pos_ps[:])

            # ---------------- outputs ----------------
            nc.sync.dma_start(acc_d[:], ACC[:])
            nc.sync.dma_start(pos_d[:], pos_sb[:])

    nc.compile()
    return nc


def get_nc():
    if "nc" not in _CACHE:
        _CACHE["nc"] = _build_nc()
    return _CACHE["nc"]


def make_in_maps(strong: np.ndarray, weak: np.ndarray, temp: np.ndarray):
    """Host-side sharding: slice + rotate + transpose (pure data movement)."""
    invtau = np.float32(1.0) / np.asarray(temp, np.float32).reshape(1, 1)
    in_maps = []
    for c in range(NCORES):
        r = c * IC
        sneg = np.roll(strong[P:B], -r, axis=0)
        wneg = np.roll(weak[P:B], -r, axis=0)
        spos = np.roll(strong[0:P], -r, axis=0)
        wp = weak[r:r + IC]
        cols = np.concatenate([sneg[0:IC], wneg[0:IC], spos, wp,
                               sneg[IC:], wneg[IC:]], axis=0)
        zt = np.ascontiguousarray(cols.T.astype(np.float16))
        in_maps.append({"zt": zt, "invtau": invtau})
    return in_maps


def kernel(inputs, strong_inputs, targets, num_pos, temperature):
    assert int(num_pos) == P
    strong = np.ascontiguousarray(np.asarray(strong_inputs, dtype=np.float32))
    weak = np.ascontiguousarray(np.asarray(inputs, dtype=np.float32))
    temp = np.asarray(temperature, dtype=np.float32).reshape(1, 1)

    from concourse.bass_utils import run_bass_kernel_spmd

    nc = get_nc()
    in_maps = make_in_maps(strong, weak, temp)
    res = run_bass_kernel_spmd(nc, in_maps, core_ids=list(range(NCORES)))
    return finish_host(res.results)


def finish_host(results):
    """Final ln(S + e^pos) - pos reduction in float64 on the host."""
    total = 0.0
    for r in results:
        acc = np.asarray(r["acc"], np.float64)      # [128, 12]
        pos = np.asarray(r["pos"], np.float64).reshape(512)
        p = np.arange(128)
        # M2 rows: ACC cols 0..3 = sn0, sn1, wn0, wn1
        for c in range(4):
            q = pos[256 + (c % 2) * 128 + p]
            total += np.sum(np.log(acc[:, c] + np.exp(q)) - q)
        # M1 rows: ACC cols 4+2t, 5+2t = the two halves of tile t
        for t in range(4):
            q = pos[(t % 2) * 128 + p]
            s = acc[:, 4 + 2 * t] + acc[:, 5 + 2 * t]
            total += np.sum(np.log(s + np.exp(q)) - q)
    return np.float32(total / (2 * B))


# revision 15
# speedup vs baseline: 1.2075x; 1.0104x over previous
"""Distributed Trainium2 kernel for nn_CompareLoss (8 NeuronCores).

Math (validated against the reference):
  z = [strong; weak]  (2B x D), s = z / ||z||  (row-normalized)
  logits(i,j) = (s_i . s_j) / tau,  pos_i = logits(i, B+i) = logits(B+i, i)
  Every row r contributes  ln(e^{pos_r} + sum_{j in C(r)} e^{logits(r,j)})
  - pos_r, with column set C(r):
    - positive rows (strong_i / weak_{B+i}, i < P): C = all 2N negative rows
    - negative rows (i >= P):                       C = the P strong-pos rows
  loss = (sum over all 2B rows) / (2B).  Logits bounded by 1/tau -> no
  max-subtraction needed in the LSE.

Sharding: data-parallel over the pair index. Core c owns 256 positive and
256 negative pairs -> 1024 rows/core.  Each core gets the full column set
feature-major (z^T, fp16) with its own row blocks rotated to the front so
one SPMD program serves all 8 cores.  No collectives; the host sums 8
tiny partial tensors.

Device pipeline (engine-balanced around the ACT exp floor of ~24us):
  - zt loads stream in 8 chunked DMAs; the M2-critical prefix
    [sn|wn|spos|wp] (2816 cols) first.
  - Column norms: DVE squares (one 3D [128,2,w] op per chunk) -> paired
    ones-matmuls whose lhsT VALUE is 1/tau (runtime, broadcast via a K=1
    matmul), accumulating invtau*ssq into a multi-partition [rows,512]
    psum tile -> DVE reciprocal -> one tiny ACT Sqrt -> rn rows, written
    flat to DRAM once and partition-broadcast back in 3+4 wide DMAs.
    This keeps ACT's non-exp work ~1.5us total (the baseline spent ~17us
    in single-partition Ln/Exp passes + 5 table swaps).
  - 12 main jobs [128,2048]: 8 fp16 matmuls + one ACT Exp with fused
    row-sum (accum_out).  The exp outputs are written to one write-only
    scratch tile; only the row sums (ACC[128,12]) leave the device.
  - Raw pos-pair logits ([1,512] psum) ship to the host as well; the
    host does the final ln(S + e^pos) - pos reduction in float64 (cheap:
    12K values/core), removing the tail Ln + its table load.
  - PE is kept continuously busy from ~6.5us (junk warm-up matmuls) so
    the tensor engine reaches its max p-state before the real matmuls.
  - DMA triggers cost ~600ns each on the issuing queue: zt + broadcast +
    output triggers all live on the Sync queue (idle otherwise); the
    Scalar queue runs pure ACT so exps are never blocked behind DMA.
  - PSUM is exactly 8 banks: every psum tile shares one 2-slot pool
    ([128,2048] slots); iv/ssqA/ssqB rotate through before the mains.
"""

import numpy as np

B = 4096
D = 256
P = 2048
NCORES = 8
IC = P // NCORES          # 256 pair-indices per core (per pos/neg half)
NCOL = 3 * P + IC         # 6400 columns

# column layout: [sn 256 | wn 256 | spos 2048 | wp 256 | snr 1792 | wnr 1792]
OFF_SN = 0
OFF_WN = 256
OFF_SPOS = 512
OFF_WP = 2560
OFF_SNR = 2816
OFF_WNR = 4608

# zt DMA chunks (start, width): M2-critical prefix first, then G2
ZT_CHUNKS = [(0, 512), (512, 1024), (1536, 1024), (2560, 256),
             (2816, 1024), (3840, 1024), (4864, 1024), (5888, 512)]
N_G1 = 4                  # first 4 chunks = G1 (cols 0:2816)

# ssq rows: 512 columns per psum partition row
G1_COLS = 2816            # ssA rows 0..5 (row5 cols 0:256 used)
G2_COLS = 3584            # ssB rows 0..6

# rn broadcast ranges (dst start, width, flat tensor, flat offset)
BCA = [(0, 1024, 0), (1024, 1024, 1024), (2048, 768, 2048)]
BCB = [(2816, 1024, 0), (3840, 1024, 1024), (4864, 1024, 2048),
       (5888, 512, 3072)]

# main jobs: (lhsT col, [4 rhs 512-col slice starts], ACC col)
M2_JOBS = [(OFF_SN, [512, 1024, 1536, 2048], 0),
           (OFF_SN + 128, [512, 1024, 1536, 2048], 1),
           (OFF_WN, [512, 1024, 1536, 2048], 2),
           (OFF_WN + 128, [512, 1024, 1536, 2048], 3)]
# the 8 negative 512-col slices: [0:512] = sn+wn, the rest = snr+wnr.
# Each M1 lhsT tile covers slice-set A in its first job and B in its
# second (order within a job is irrelevant - only the row-sum is kept).
NEG_A = [0, 2816, 3328, 3840]
NEG_B = [4352, 4864, 5376, 5888]
M1_JOBS = [(OFF_SPOS, NEG_A, 4),
           (OFF_SPOS + 128, NEG_A, 6),
           (OFF_WP, NEG_A, 8),
           (OFF_WP + 128, NEG_A, 10)]
M1_JOBS_2 = [(OFF_SPOS, NEG_B, 5),
             (OFF_SPOS + 128, NEG_B, 7),
             (OFF_WP, NEG_B, 9),
             (OFF_WP + 128, NEG_B, 11)]

_CACHE: dict = {}


def _build_nc():
    import concourse.bacc as bacc
    import concourse.tile as tile
    from concourse import mybir
    from concourse.tile_rust import add_dep_helper

    f32 = mybir.dt.float32
    f16 = mybir.dt.float16
    EXP = mybir.ActivationFunctionType.Exp
    SQRT = mybir.ActivationFunctionType.Sqrt

    nc = bacc.Bacc("TRN2", target_bir_lowering=False, debug=False,
                   num_devices=NCORES)
    zt_d = nc.dram_tensor("zt", [2 * D // 2, NCOL], f16, kind="ExternalInput")
    iv_d = nc.dram_tensor("tauv", [1, 1], f32, kind="ExternalInput")
    acc_d = nc.dram_tensor("acc", [128, 12], f32, kind="ExternalOutput")
    pos_d = nc.dram_tensor("pos", [1, 512], f32, kind="ExternalOutput")

    zt3 = zt_d[:, :].rearrange("(h p) c -> p h c", h=2)  # [128,2,NCOL] view

    with tile.TileContext(nc) as tc:
        with (
            tc.tile_pool(name="const", bufs=1) as constp,
            tc.tile_pool(name="big", bufs=1) as bigp,
            tc.tile_pool(name="work", bufs=3) as workp,
            tc.tile_pool(name="dram", bufs=1, space="DRAM") as dramp,
            tc.tile_pool(name="ps", bufs=2, space="PSUM") as psp,
        ):
            # ---------------- input DMAs (sync queue) ----------------
            zt = bigp.tile([128, 2, NCOL], f16)
            ivt = constp.tile([1, 1], f32)
            d_iv = nc.sync.dma_start(ivt[:], iv_d[:])
            prev = [d_iv]
            for s, w in ZT_CHUNKS:
                d = nc.sync.dma_start(zt[:, :, s:s + w], zt3[:, :, s:s + w])
                # chain with depth 2 so chunks arrive in order at full
                # bandwidth instead of round-robining to a joint finish
                if len(prev) >= 2:
                    add_dep_helper(d.ins, prev[-2].ins, sync=True,
                                   reason="zt chunk ordering")
                prev.append(d)

            # ---------------- constants ----------------
            ones16_1 = constp.tile([1, 128], f16)
            nc.gpsimd.memset(ones16_1[:], 1.0)
            ones16_k = constp.tile([128, 1], f16)
            nc.gpsimd.memset(ones16_k[:], 1.0)
            junkW = constp.tile([128, 128], f16)
            nc.gpsimd.memset(junkW[:], 0.0)
            junkR = constp.tile([128, 1], f16)
            nc.gpsimd.memset(junkR[:], 0.0)

            # tau broadcast to [128,1] via K=1 matmul.  EZ is the
            # sumsq lhsT: sliding 13-col windows of [0*13 | tau | 0*12]
            # place tau*ssq of chunk r at psum partition r (recip then
            # gives invtau/ssq, sqrt of that is the rn scale; matmul
            # output must start at partition 0, so the row index comes
            # from the hot column's position inside the window).
            iv16 = constp.tile([1, 1], f16)
            nc.vector.tensor_copy(iv16[:], ivt[:])
            iv_ps = psp.tile([128, 1], f32, tag="ps")
            nc.tensor.matmul(iv_ps[:], ones16_1[0:1, :], iv16[0:1, 0:1],
                             start=True, stop=True)
            EZ = constp.tile([128, 26], f16)
            nc.gpsimd.memset(EZ[:], 0.0)
            nc.vector.tensor_copy(EZ[:, 13:14], iv_ps[:])

            # PE warm-up: keep the tensor engine continuously busy so it
            # ramps to max p-state before the real matmuls (ldweights of
            # a [128,128] tile dominates; results overwrite iv_ps, dead)
            for _ in range(12):
                nc.tensor.matmul(iv_ps[:, 0:1], junkW[:], junkR[0:128, 0:1],
                                 start=True, stop=True)

            # ---------------- column sumsq -> rn ----------------
            # squares per chunk (one 3D DVE op), then EZ-window matmuls
            # accumulating invtau*ssq of 512-col subchunk r into psum
            # partition r of a single [13,512] tile (1 bank).  The first
            # matmul (start=True, full width) zero-fills all rows, so the
            # reciprocal never reads uninitialized psum.
            ssA = psp.tile([13, 512], f32, tag="ps")
            ssB = psp.tile([13, 512], f32, tag="ps")

            def do_chunk(s, w, grp_base, ss, first, last):
                sq = workp.tile([128, 2, 1024], f16, tag="sq",
                                name=f"sq_{s}")
                nc.vector.tensor_mul(sq[:, :, 0:w], zt[:, :, s:s + w],
                                     zt[:, :, s:s + w])
                lo = 0
                while lo < w:
                    r = (s - grp_base + lo) // 512
                    ww = min(512, w - lo)
                    for h in range(2):
                        nc.tensor.matmul(
                            ss[0:13, 0:ww], EZ[:, 13 - r:26 - r],
                            sq[:, h, lo:lo + ww],
                            start=(first and lo == 0 and h == 0),
                            stop=(last and lo + ww >= w and h == 1))
                    lo += ww

            for i, (s, w) in enumerate(ZT_CHUNKS[:N_G1]):
                do_chunk(s, w, 0, ssA, i == 0, i == N_G1 - 1)

            # G1 rn: DVE reciprocal -> ACT sqrt -> f16 rows
            rsqA = constp.tile([6, 512], f32)
            nc.vector.reciprocal(rsqA[:], ssA[0:6, :])
            rn_tA = constp.tile([6, 512], f16)
            nc.scalar.activation(rn_tA[:], rsqA[:], SQRT)
            flatA = dramp.tile([1, 3072], f16)
            d_fA = nc.sync.dma_start(
                flatA[0:1, :].rearrange("o (p c) -> p (o c)", p=6), rn_tA[:])

            # G2 chunks (squares + matmuls run as data lands)
            ng2 = len(ZT_CHUNKS) - N_G1
            for i, (s, w) in enumerate(ZT_CHUNKS[N_G1:]):
                do_chunk(s, w, OFF_SNR, ssB, i == 0, i == ng2 - 1)

            rnb = bigp.tile([128, NCOL], f16)
            bcA = []
            for ds, w, fo in BCA:
                d = nc.sync.dma_start(rnb[:, ds:ds + w],
                                      flatA[0:1, fo:fo + w]
                                      .to_broadcast((128, w)))
                add_dep_helper(d.ins, d_fA.ins, sync=True,
                               reason="bcast after flat write")
                bcA.append(d)

            # G1 normalize (DVE), lhsT/pos chunks first via range order
            ztn = bigp.tile([128, 2, NCOL], f16)
            for ds, w, _ in BCA:
                for h in range(2):
                    nc.vector.tensor_mul(ztn[:, h, ds:ds + w],
                                         zt[:, h, ds:ds + w],
                                         rnb[:, ds:ds + w])

            # G2 rn
            rsqB = constp.tile([7, 512], f32)
            nc.vector.reciprocal(rsqB[:], ssB[0:7, :])
            rn_tB = constp.tile([7, 512], f16)
            nc.scalar.activation(rn_tB[:], rsqB[:], SQRT)
            flatB = dramp.tile([1, 3584], f16)
            d_fB = nc.sync.dma_start(
                flatB[0:1, :].rearrange("o (p c) -> p (o c)", p=7), rn_tB[:])
            for ds, w, fo in BCB:
                d = nc.sync.dma_start(rnb[:, ds:ds + w],
                                      flatB[0:1, fo:fo + w]
                                      .to_broadcast((128, w)))
                add_dep_helper(d.ins, d_fB.ins, sync=True,
                               reason="bcast after flat write")

            # ---------------- main similarity jobs ----------------
            ACC = constp.tile([128, 12], f32)
            esc = constp.tile([128, 2048], f16)   # write-only exp sink

            def main_job(lhs_off, rhs_list, acccol):
                ps = psp.tile([128, 2048], f32, tag="ps",
                              name=f"mm{acccol}")
                for h4, c0 in enumerate(rhs_list):
                    for h in range(2):
                        nc.tensor.matmul(
                            ps[:, h4 * 512:(h4 + 1) * 512],
                            ztn[:, h, lhs_off:lhs_off + 128],
                            ztn[:, h, c0:c0 + 512],
                            start=(h == 0), stop=(h == 1))
                nc.scalar.activation(esc[:], ps[:], EXP,
                                     accum_out=ACC[:, acccol:acccol + 1])

            for lhs_off, rhs_list, acccol in M2_JOBS:
                main_job(lhs_off, rhs_list, acccol)

            # G2 normalize interleaves with the M2 jobs on the DVE
            for ds, w, _ in BCB:
                for h in range(2):
                    nc.vector.tensor_mul(ztn[:, h, ds:ds + w],
                                         zt[:, h, ds:ds + w],
                                         rnb[:, ds:ds + w])

            for lhs_off, rhs_list, acccol in M1_JOBS:
                main_job(lhs_off, rhs_list, acccol)
            for lhs_off, rhs_list, acccol in M1_JOBS_2:
                main_job(lhs_off, rhs_list, acccol)

            # ---------------- raw pos-pair logits ----------------
            # products of normalized columns; summed over k by ones-matmul
            pr_pos = workp.tile([128, 2, 256], f16, tag="pr")
            nc.vector.tensor_mul(pr_pos[:],
                                 ztn[:, :, OFF_SPOS:OFF_SPOS + 256],
                                 ztn[:, :, OFF_WP:OFF_WP + 256])
            pr_neg = workp.tile([128, 2, 256], f16, tag="pr")
            nc.vector.tensor_mul(pr_neg[:],
                                 ztn[:, :, OFF_SN:OFF_SN + 256],
                                 ztn[:, :, OFF_WN:OFF_WN + 256])
            pos_ps = psp.tile([1, 512], f32, tag="ps")
            for half, pr in ((0, pr_pos), (1, pr_neg)):
                o = half * 256
                nc.tensor.matmul(pos_ps[0:1, o:o + 256], ones16_k[:],
                                 pr[:, 0, :], start=True, stop=False)
                nc.tensor.matmul(pos_ps[0:1, o:o + 256], ones16_k[:],
                                 pr[:, 1, :], start=False, stop=True)
            pos_sb = constp.tile([1, 512], f32)
            nc.vector.tensor_copy(pos_sb[:], pos_ps[:])

            # ---------------- outputs ----------------
            nc.sync.dma_start(acc_d[:], ACC[:])
            nc.sync.dma_start(pos_d[:], pos_sb[:])

    nc.compile()
    return nc


def get_nc():
    if "nc" not in _CACHE:
        _CACHE["nc"] = _build_nc()
    return _CACHE["nc"]


def make_in_maps(strong: np.ndarray, weak: np.ndarray, temp: np.ndarray):
    """Host-side sharding: slice + rotate + transpose (pure data movement)."""
    tauv = np.asarray(temp, np.float32).reshape(1, 1)
    in_maps = []
    for c in range(NCORES):
        r = c * IC
        sneg = np.roll(strong[P:B], -r, axis=0)
        wneg = np.roll(weak[P:B], -r, axis=0)
        spos = np.roll(strong[0:P], -r, axis=0)
        wp = weak[r:r + IC]
        cols = np.concatenate([sneg[0:IC], wneg[0:IC], spos, wp,
                               sneg[IC:], wneg[IC:]], axis=0)
        zt = np.ascontiguousarray(cols.T.astype(np.float16))
        in_maps.append({"zt": zt, "tauv": tauv})
    return in_maps


def kernel(inputs, strong_inputs, targets, num_pos, temperature):
    assert int(num_pos) == P
    strong = np.ascontiguousarray(np.asarray(strong_inputs, dtype=np.float32))
    weak = np.ascontiguousarray(np.asarray(inputs, dtype=np.float32))
    temp = np.asarray(temperature, dtype=np.float32).reshape(1, 1)

    from concourse.bass_utils import run_bass_kernel_spmd

    nc = get_nc()
    in_maps = make_in_maps(strong, weak, temp)
    res = run_bass_kernel_spmd(nc, in_maps, core_ids=list(range(NCORES)))
    return finish_host(res.results)


def finish_host(results):
    """Final ln(S + e^pos) - pos reduction in float64 on the host."""
    total = 0.0
    for r in results:
        acc = np.asarray(r["acc"], np.float64)      # [128, 12]
        pos = np.asarray(r["pos"], np.float64).reshape(512)
        p = np.arange(128)
        # M2 rows: ACC cols 0..3 = sn0, sn1, wn0, wn1
        for c in range(4):
            q = pos[256 + (c % 2) * 128 + p]
            total += np.sum(np.log(acc[:, c] + np.exp(q)) - q)
        # M1 rows: ACC cols 4+2t, 5+2t = the two halves of tile t
        for t in range(4):
            q = pos[(t % 2) * 128 + p]
            s = acc[:, 4 + 2 * t] + acc[:, 5 + 2 * t]
            total += np.sum(np.log(s + np.exp(q)) - q)
    return np.float32(total / (2 * B))
